# revision 43
# baseline (speedup 1.0000x reference)
"""CrossTemporalAttention2 Trainium2 kernel (v3).

Sharding: 8 cores = 2 batches x 4 query-chunks of 1024 rows. Each core runs
the full conv+LN+KV pipeline for its batch (duplicated within the batch
group) and attention + proj for its 1024 query rows.

v3 restructuring vs v2:
- Contiguous w2 host layout (was a strided on-DMA rearrange emitting ~1k
  descriptors that serialized the scalar queue and delayed first compute to
  ~28us).
- Load priority: wall/w2 then x2 then x1 split across the two HWDGE queues
  (sync + scalar); xq + small consts on the gpsimd SWDGE queue. Compute
  starts as soon as its inputs land (~5us).
- sq (xr^2 for variance) moved from Pool to DVE (bf16 4x mode).
- Phases 2b/2c merged: U/pden(nh2=0) front-loaded 2-per-step during the
  nh2=1 scores loop, normalize_proj(0) mid-stream, U/pden(nh2=1) drains
  right behind the nh2=1 exps, shrinking the PE-only tail.
- Output stores on the idle sync queue.
"""

import numpy as np

B, N, C = 2, 4096, 256
H, Dh = 8, 32
M = 1024          # (64/2) * (64/2)
NCH = 1024        # query rows per core
SCALE = Dh ** -0.5
EPS = 1e-5

# minimax cubic fit of exp on [-0.95, 0.95] (rel err <= 4.1e-3 fp32)
C3, C2, C1, C0 = 0.15927659, 0.53526688, 1.00884709, 0.99703789

# number of exp tiles (of 64) handled by the DVE cubic; rest on ACT
DVE_TILES = 11

_prog_cache = {}


def _exp_engine(t):
    # t in [0, 64): tile index in emission order. Returns 'act'|'dve'.
    # First/last tiles forced to ACT (fast path): early the pipeline is
    # shallow, late it must drain.
    lo, hi = 6, 58
    if t < lo or t >= hi:
        return "act"
    tt, n = t - lo, hi - lo
    if (tt * DVE_TILES) // n != ((tt + 1) * DVE_TILES) // n:
        return "dve"
    return "act"


def _build_program():
    import concourse.bass as bass
    import concourse.bacc as bacc
    import concourse.tile as tile
    from concourse import mybir

    f32 = mybir.dt.float32
    bf16 = mybir.dt.bfloat16
    AF = mybir.ActivationFunctionType
    OP = mybir.AluOpType

    nc = bacc.Bacc()

    x1t = nc.dram_tensor("x1t", [C, N], bf16, kind="ExternalInput")
    x2t = nc.dram_tensor("x2t", [C, N], bf16, kind="ExternalInput")
    w2d = nc.dram_tensor("w2", [2, 128, 2, 2, C], bf16, kind="ExternalInput")
    wall = nc.dram_tensor("wall", [C, 4 * C], bf16, kind="ExternalInput")
    rowd = nc.dram_tensor("rowd", [3, C], bf16, kind="ExternalInput")   # -kfcol, -vfcol, pbias
    cold = nc.dram_tensor("cold", [128, 4], f32, kind="ExternalInput")  # srb(2 oh), pb(2 oh)
    blkd = nc.dram_tensor("blkd", [2, 128, 128], bf16, kind="ExternalInput")
    outt = nc.dram_tensor("outt", [C, NCH], bf16, kind="ExternalOutput")

    with nc.allow_low_precision(reason="bf16 matmul inputs; fp32 PSUM accumulation"), \
         tile.TileContext(nc) as tc:
      with tc.tile_pool(name="pg", bufs=1) as PG, \
           tc.tile_pool(name="psum", bufs=1, space="PSUM") as PS:
        # ================= consts / weights =================
        # Big loads ride the two HWDGE queues (sync + scalar), interleaved
        # by consumption order; xq + tiny consts go on the gpsimd SWDGE
        # queue so q never queues behind the x loads.
        wallt = [PG.tile([128, 4 * C], bf16, name=f"wall{ch}",
                         tag=f"wall{ch}") for ch in range(2)]
        w2 = [PG.tile([128, 2, 2, C], bf16, name=f"w2{ch}", tag=f"w2{ch}")
              for ch in range(2)]
        xT = {(inp, ch): PG.tile([128, N], bf16, name=f"x{inp}{ch}",
                                 tag=f"x{inp}{ch}")
              for inp in (1, 0) for ch in range(2)}
        # x1/x2 are staged column-rotated per core so this core's q rows are
        # always x1 columns [0:1024); xq is just a view of the x1 tile.
        xq = [xT[(0, ch)][:, 0:NCH] for ch in range(2)]

        # HWDGE queues (ch0 on sync, ch1 on scalar), pieces in exact
        # consumption order: conv(x2 h0) -> k2x (wall k cols) -> q (wall q
        # cols + x1[0:512]) -> q nh=1 -> conv(x1 h0) -> conv(x2 h1) ->
        # conv(x1 h1).
        dmae = [nc.sync, nc.scalar]
        for ch in range(2):
            rs = slice(ch * 128, (ch + 1) * 128)
            e = dmae[ch]
            e.dma_start(out=w2[ch], in_=w2d[ch])
            e.dma_start(out=wallt[ch][:, 0:256], in_=wall[rs, 0:256])
            e.dma_start(out=xT[(1, ch)][:, 0:2048], in_=x2t[rs, 0:2048])
            e.dma_start(out=wallt[ch][:, 256:512], in_=wall[rs, 256:512])
            e.dma_start(out=xT[(0, ch)][:, 0:512], in_=x1t[rs, 0:512])
            e.dma_start(out=wallt[ch][:, 512:1024], in_=wall[rs, 512:1024])
            e.dma_start(out=xT[(0, ch)][:, 512:1024], in_=x1t[rs, 512:1024])
            e.dma_start(out=xT[(0, ch)][:, 1024:2048], in_=x1t[rs, 1024:2048])
            e.dma_start(out=xT[(1, ch)][:, 2048:4096], in_=x2t[rs, 2048:4096])
            e.dma_start(out=xT[(0, ch)][:, 2048:4096], in_=x1t[rs, 2048:4096])

        cols = PG.tile([128, 4], f32, name="cols", tag="cols")
        nc.gpsimd.dma_start(out=cols, in_=cold[:])
        kfneg = PG.tile([1, C], bf16, name="kfneg", tag="kfneg")
        nc.gpsimd.dma_start(out=kfneg, in_=rowd[0:1, :])
        vfneg = PG.tile([1, C], bf16, name="vfneg", tag="vfneg")
        nc.gpsimd.dma_start(out=vfneg, in_=rowd[1:2, :])
        pbrow = PG.tile([1, C], bf16, name="pbrow", tag="pbrow")
        nc.gpsimd.dma_start(out=pbrow, in_=rowd[2:3, :])
        blk = []
        for grp in range(2):
            t = PG.tile([128, 128], bf16, name=f"blk{grp}", tag=f"blk{grp}")
            nc.gpsimd.dma_start(out=t, in_=blkd[grp])
            blk.append(t)

        wsb = {}
        for wi, nm in enumerate(("q", "k", "v", "p")):
            for ch in range(2):
                wsb[(nm, ch)] = wallt[ch][:, wi * C:(wi + 1) * C]

        ones1 = PG.tile([1, 128], bf16, name="ones1", tag="ones1")
        nc.vector.memset(ones1, 1.0)
        ones512 = PG.tile([1, 512], bf16, name="ones512", tag="ones512")
        nc.vector.memset(ones512, 1.0)
        ones11 = PG.tile([1, 1], f32, name="ones11", tag="ones11")
        nc.vector.memset(ones11, 1.0)
        selc = PG.tile([128, 1], bf16, name="selc", tag="selc")
        nc.vector.memset(selc, 1.0)
        sel8 = PG.tile([128, 2, 8], bf16, name="sel8", tag="sel8")
        nc.vector.memset(sel8, 0.0)
        for g in range(2):
            nc.vector.memset(sel8[:, g, g:g + 1], 1.0)
        epsT = PG.tile([1, 1], f32, name="epsT", tag="epsT")
        nc.vector.memset(epsT, EPS)

        # ================= SBUF data tiles =================
        qT = [PG.tile([128, NCH], bf16, name=f"qT{oh}", tag=f"qT{oh}")
              for oh in range(2)]
        xr = {(inp, oh): PG.tile([128, M], bf16, name=f"xr{inp}{oh}",
                                 tag=f"xr{inp}{oh}")
              for inp in range(2) for oh in range(2)}
        # sq tiles: 2 bufs per tag so the conv(x1) squares can be emitted
        # before the x2 stats consumer without a cross-engine cycle
        sq = {}
        for inp in (1, 0):   # allocation order must match usage order
            for oh in range(2):
                sq[(inp, oh)] = PG.tile([128, M], bf16, name=f"sqt{oh}",
                                        tag=f"sqt{oh}", bufs=2)
        k2x = [PG.tile([128, M], bf16, name=f"k2x{oh}", tag=f"k2x{oh}")
               for oh in range(2)]
        v1 = [PG.tile([128, C], bf16, name=f"v1_{ms}", tag=f"v1_{ms}")
              for ms in range(8)]
        murow = [PG.tile([1, M], bf16, name=f"mu{inp}", tag=f"mu{inp}")
                 for inp in range(2)]
        varrow = [PG.tile([1, M], f32, name=f"va{inp}", tag=f"va{inp}")
                  for inp in range(2)]
        rcol = [PG.tile([128, 8], f32, name=f"rcol{inp}", tag=f"rcol{inp}")
                for inp in range(2)]

        def cvt():
            return PS.tile([128, 512], f32, name="cv", tag="cv", bufs=2)

        # ================= q (cv ring; interleaved with conv below) ====
        def q_tile(oh, nh):
            ps = PS.tile([128, 512], f32, name="qp", tag="cv", bufs=2)
            for ch in range(2):
                nc.tensor.matmul(
                    ps, wsb[("q", ch)][:, oh * 128:(oh + 1) * 128],
                    xq[ch][:, nh * 512:(nh + 1) * 512],
                    start=(ch == 0), stop=(ch == 1))
            nc.scalar.copy(out=qT[oh][:, nh * 512:(nh + 1) * 512], in_=ps)

        # ================= conv + stats + proj-k/v helpers ============
        def conv_split(inp, mh, oh):
            # conv for m-columns [mh*512, (mh+1)*512) of one oh chunk,
            # split into two 4-matmul halves so it interleaves finely with
            # the scores stream (PSUM accumulation stays open in between).
            ps = cvt()

            def half(h):
                for k in range(h * 4, h * 4 + 4):
                    ch, kh, kw = k // 4, (k // 2) % 2, k % 2
                    xv = xT[(inp, ch)].rearrange(
                        "p (i ki j kj) -> p ki kj i j", ki=2, kj=2, j=32)
                    nc.tensor.matmul(
                        ps,
                        w2[ch][:, kh, kw, oh * 128:(oh + 1) * 128],
                        xv[:, kh, kw, mh * 16:(mh + 1) * 16, :],
                        start=(k == 0), stop=(k == 7))
                if h == 1:
                    sl = slice(mh * 512, (mh + 1) * 512)
                    # copy + per-partition conv bias (srb col 0/1)
                    nc.vector.tensor_scalar(
                        out=xr[(inp, oh)][:, sl], in0=ps,
                        scalar1=cols[:, oh:oh + 1], scalar2=None, op0=OP.add)
                    nc.vector.tensor_mul(sq[(inp, oh)][:, sl],
                                         xr[(inp, oh)][:, sl],
                                         xr[(inp, oh)][:, sl])
            return half

        def conv_tile(inp, mh, oh):
            h = conv_split(inp, mh, oh)
            h(0)
            h(1)

        def stats_half(inp, mh):
            sl = slice(mh * 512, (mh + 1) * 512)
            pmu = PS.tile([1, 512], f32, name="pmu", tag="cv", bufs=2)
            psq = PS.tile([1, 512], f32, name="psq", tag="cv", bufs=2)
            for k, oh in enumerate(range(2)):
                nc.tensor.matmul(pmu, selc, xr[(inp, oh)][:, sl],
                                 start=(k == 0), stop=(k == 1))
                nc.tensor.matmul(psq, selc, sq[(inp, oh)][:, sl],
                                 start=(k == 0), stop=(k == 1))
            nc.scalar.mul(out=murow[inp][:, sl], in_=pmu, mul=1.0 / C)
            mu2 = PG.tile([1, 512], f32, name="mu2", tag="mu2", bufs=2)
            nc.vector.tensor_mul(mu2, murow[inp][:, sl], murow[inp][:, sl])
            nc.vector.scalar_tensor_tensor(
                out=varrow[inp][:, sl], in0=psq, scalar=1.0 / C, in1=mu2,
                op0=OP.mult, op1=OP.subtract)

        def rcol_make(inp, mh):
            # var row half [1, 512] -> [128, 4] columns via four K=1 PE
            # matmuls (a transpose: out[m,0] = var[0,m]), then
            # rsqrt(var + eps) on DVE: bit-trick seed + 2 Newton steps.
            # (Replaces a DRAM round-trip that blocked the in-order DVE
            # queue behind DMA latency.)
            cs = slice(mh * 4, (mh + 1) * 4)
            vps = PS.tile([128, 4], f32, name="vps", tag="cv", bufs=2)
            for j in range(4):
                nc.tensor.matmul(
                    vps[:, j:j + 1],
                    varrow[inp][:, mh * 512 + 128 * j:mh * 512 + 128 * (j + 1)],
                    ones11, start=True, stop=True, skip_group_check=True)
            vc = PG.tile([128, 4], f32, name="vc", tag="vc", bufs=2)
            nc.vector.tensor_scalar(out=vc, in0=vps, scalar1=float(EPS),
                                    scalar2=None, op0=OP.add)
            i32 = mybir.dt.int32
            sh = PG.tile([128, 4], i32, name="sh", tag="sh", bufs=2)
            nc.vector.tensor_scalar(out=sh, in0=vc.bitcast(i32), scalar1=1,
                                    scalar2=None, op0=OP.arith_shift_right)
            y0 = PG.tile([128, 4], i32, name="y0", tag="y0", bufs=2)
            nc.vector.tensor_scalar(out=y0, in0=sh, scalar1=-1,
                                    scalar2=0x5F3759DF,
                                    op0=OP.mult, op1=OP.add)
            y = y0.bitcast(f32)
            for it in range(2):
                c = PG.tile([128, 4], f32, name="nc1", tag="nc1", bufs=2)
                nc.vector.tensor_mul(c, y, y)
                nc.vector.tensor_mul(c, c, vc)
                nc.vector.tensor_scalar(out=c, in0=c, scalar1=-0.5,
                                        scalar2=1.5, op0=OP.mult, op1=OP.add)
                dst = rcol[inp][:, cs] if it == 1 else y
                nc.vector.tensor_mul(dst, y, c)

        def k2x_part(mh, oh):
            sl = slice(mh * 512, (mh + 1) * 512)
            ps = cvt()
            for ch in range(2):
                nc.tensor.matmul(
                    ps, wsb[("k", ch)][:, oh * 128:(oh + 1) * 128],
                    xr[(1, ch)][:, sl], start=(ch == 0), stop=False)
            # rank-1: += (-kfcol) x mu2  (mean subtraction folded in)
            nc.tensor.matmul(
                ps, kfneg[:, oh * 128:(oh + 1) * 128],
                murow[1][:, sl], start=False, stop=True)
            nc.vector.tensor_copy(out=k2x[oh][:, sl], in_=ps)

        def k2x_half(mh):
            for oh in range(2):
                k2x_part(mh, oh)

        def v1_chunk(ms):
            msl = slice(ms * 128, (ms + 1) * 128)
            ps = PS.tile([128, C], f32, name="vp", tag="cv", bufs=2)
            for ch in range(2):
                nc.tensor.matmul(ps, xr[(0, ch)][:, msl], wsb[("v", ch)],
                                 start=(ch == 0), stop=False)
            # rank-1: += mu1 x (-vfcol)
            nc.tensor.matmul(ps, murow[0][:, msl], vfneg,
                             start=False, stop=True)
            # copy with per-partition rstd scale folded in, on ACT (keeps
            # the DVE queue free for the exp cubics)
            nc.scalar.mul(out=v1[ms], in_=ps, mul=rcol[0][:, ms:ms + 1])

        # ================= phase 2 machinery =================
        ET = tc.alloc_tile_pool(name="et", bufs=36)
        XG = tc.alloc_tile_pool(name="xg", bufs=3)
        tile_ctr = [0]

        def scores_tile(nh2, ms, grp, pr):
            """scores for one pr-tile (2 heads) + exp on assigned engine."""
            nsl = slice(nh2 * 512, (nh2 + 1) * 512)
            t = tile_ctr[0]
            tile_ctr[0] += 1
            eng = _exp_engine(t)
            scps = PS.tile([128, 1024], f32, name="scps", tag="scA",
                           bufs=2)
            for i in range(2):
                h = grp * 4 + pr * 2 + i
                hb = 32 * (h % 4)
                nc.tensor.matmul(
                    scps[:, i * 512:(i + 1) * 512],
                    k2x[h // 4][hb:hb + 32, ms * 128:(ms + 1) * 128],
                    qT[h // 4][hb:hb + 32, nsl],
                    start=True, stop=True,
                    tile_position=(hb, 0))
            et = ET.tile([128, 1024], bf16, name="et", tag="et")
            rsc = rcol[1][:, ms:ms + 1]
            if eng == "act":
                nc.scalar.activation(out=et, in_=scps, func=AF.Exp,
                                     scale=rsc)
            else:  # dve cubic
                t = XG.tile([128, 1024], bf16, name="t", tag="t", bufs=1)
                nc.vector.tensor_scalar(
                    out=t, in0=scps, scalar1=rsc, scalar2=None,
                    op0=OP.mult)
                u = XG.tile([128, 1024], bf16, name="u", tag="u", bufs=1)
                nc.vector.tensor_mul(u, t, t)
                v = XG.tile([128, 1024], bf16, name="v", tag="v", bufs=1)
                nc.vector.tensor_scalar(
                    out=v, in0=t, scalar1=float(C3), scalar2=float(C2),
                    op0=OP.mult, op1=OP.add)
                p = XG.tile([128, 1024], bf16, name="p", tag="p", bufs=1)
                nc.vector.tensor_mul(p, u, v)
                w = XG.tile([128, 1024], bf16, name="w", tag="w", bufs=1)
                nc.vector.tensor_scalar(
                    out=w, in0=t, scalar1=float(C1), scalar2=float(C0),
                    op0=OP.mult, op1=OP.add)
                nc.vector.tensor_add(et, w, p)
            return et

        U = {}
        pden = {}

        def upden_U(nh2, ms, grp, ets):
            if ms == 0 and grp == 0:
                U[(nh2, 0)] = PS.tile([128, 512], f32, name="U0", tag="U0")
                U[(nh2, 1)] = PS.tile([128, 512], f32, name="U1", tag="U1")
            for pr in range(2):
                for i in range(2):
                    h = grp * 4 + pr * 2 + i
                    h4 = pr * 2 + i
                    esl = ets[pr][:, i * 512:(i + 1) * 512]
                    nc.tensor.matmul(
                        U[(nh2, grp)][32 * h4:32 * h4 + 32, :],
                        v1[ms][:, 32 * h:32 * h + 32], esl,
                        start=(ms == 0), stop=(ms == 7),
                        tile_position=(0, 32 * h4),
                        skip_group_check=True)

        def upden_P(nh2, ms, grp, ets):
            if ms == 0 and grp == 0:
                pden[nh2] = PS.tile([128, 512], f32, name="pden", tag="cv",
                                    bufs=2)
            for pr in range(2):
                for i in range(2):
                    h = grp * 4 + pr * 2 + i
                    g = h % 4
                    esl = ets[pr][:, i * 512:(i + 1) * 512]
                    nc.tensor.matmul(
                        pden[nh2][32 * g:32 * g + 8, :],
                        sel8[:, h // 4, :], esl,
                        start=(ms == 0 and grp == 0),
                        stop=(ms == 7 and grp == 1),
                        tile_position=(0, 32 * g),
                        skip_group_check=True)

        def upden(nh2, ms, grp, ets):
            upden_U(nh2, ms, grp, ets)
            upden_P(nh2, ms, grp, ets)

        recf = {}

        def normalize_A(nh2):
            # denominator half: pden -> bf16 -> per-head broadcast -> recip
            pdenS = PG.tile([128, 512], bf16, name="pdenS", tag="pdenS",
                            bufs=2)
            nc.vector.tensor_copy(out=pdenS, in_=pden[nh2])
            rps = PS.tile([128, 1024], f32, name="rps", tag="scA", bufs=2)
            for grp in range(2):
                nc.tensor.matmul(rps[:, grp * 512:(grp + 1) * 512],
                                 blk[grp], pdenS, start=True, stop=True)
            recf[nh2] = PG.tile([128, 1024], f32, name="recf", tag="recf",
                                bufs=2)
            nc.vector.reciprocal_approx_fast(out=recf[nh2], in_=rps)

        def normalize_B(nh2):
            nsl = slice(nh2 * 512, (nh2 + 1) * 512)
            oT = []
            for grp in range(2):
                ot = PG.tile([128, 512], bf16, name="ot", tag=f"ot{grp}",
                             bufs=2)
                nc.vector.tensor_mul(ot, U[(nh2, grp)],
                                     recf[nh2][:, grp * 512:(grp + 1) * 512])
                oT.append(ot)
            for oh in range(2):
                ps = cvt()
                for ch in range(2):
                    nc.tensor.matmul(
                        ps, wsb[("p", ch)][:, oh * 128:(oh + 1) * 128],
                        oT[ch], start=(ch == 0), stop=False)
                # proj bias as a rank-1 accumulate so the PSUM->SBUF copy
                # below is a plain copy (runs on ACT, idle at the tail)
                nc.tensor.matmul(
                    ps, pbrow[:, oh * 128:(oh + 1) * 128], ones512,
                    start=False, stop=True)
                y = PG.tile([128, 512], bf16, name="y", tag="y", bufs=2)
                nc.scalar.copy(out=y, in_=ps)
                nc.sync.dma_start(out=outt[oh * 128:(oh + 1) * 128, nsl],
                                  in_=y)

        def normalize_proj(nh2):
            normalize_A(nh2)
            normalize_B(nh2)

        # ================= emission =================
        # PE warm-up: dummy matmuls on memset tiles while the x loads are in
        # flight, so the PE p-state is ramped when real work arrives.
        warm = PG.tile([128, 512], bf16, name="warm", tag="warm")
        nc.vector.memset(warm, 0.0)
        for _ in range(26):
            wps = PS.tile([1, 512], f32, name="wps", tag="cv", bufs=2)
            nc.tensor.matmul(wps, selc, warm, start=True, stop=True)

        # phase 1a-early: only what the first scores need — conv(x2 h0) ->
        # stats -> rcol -> k2x half 0 oh 0 + q oh 0 (grp-0 scores need only
        # the oh=0 halves); the oh=1 halves follow behind the first scores.
        conv_tile(1, 0, 0)
        conv_tile(1, 0, 1)
        stats_half(1, 0)
        rcol_make(1, 0)
        k2x_part(0, 0)
        q_tile(0, 0)

        ets = {}      # (nh2, ms, grp) -> [et_pr0, et_pr1]

        def upden0_U_step(j):
            upden_U(0, j // 2, j % 2, ets[(0, j // 2, j % 2)])

        def upden0_P_step(j):
            upden_P(0, j // 2, j % 2, ets[(0, j // 2, j % 2)])

        def upden1_pair(j):
            upden(1, j // 2, j % 2, ets[(1, j // 2, j % 2)])

        # 64 units, one et tile each (unit u: nh2=u//32, then (ms, grp, pr)
        # in order). Each unit carries <=1us of extra PE "filler" work,
        # emitted AFTER its score matmuls so the exp feed is never delayed;
        # fillers are placed at the unit where their inputs have landed.
        cs110 = conv_split(1, 1, 0)
        cs111 = conv_split(1, 1, 1)
        cs000 = conv_split(0, 0, 0)
        cs001 = conv_split(0, 0, 1)
        cs010 = conv_split(0, 1, 0)
        cs011 = conv_split(0, 1, 1)
        fillers = {
            0: [lambda: k2x_part(0, 1)],
            1: [lambda: q_tile(1, 0)],
            2: [lambda: q_tile(0, 1)],
            3: [lambda: q_tile(1, 1)],
            4: [lambda: cs110(0)], 5: [lambda: cs110(1)],
            6: [lambda: cs111(0)], 7: [lambda: cs111(1)],
            8: [lambda: stats_half(1, 1)],
            9: [lambda: rcol_make(1, 1)],
            10: [lambda: k2x_part(1, 0)],
            11: [lambda: k2x_part(1, 1)],
            12: [lambda: cs000(0)], 13: [lambda: cs000(1)],
            14: [lambda: cs001(0)], 15: [lambda: cs001(1)],
            16: [lambda: (stats_half(0, 0), rcol_make(0, 0))],
            17: [lambda: v1_chunk(0)],
            18: [lambda: v1_chunk(1)],
            19: [lambda: v1_chunk(2)],
            20: [lambda: cs010(0)], 21: [lambda: cs010(1)],
            22: [lambda: cs011(0)], 23: [lambda: cs011(1)],
            24: [lambda: (stats_half(0, 1), rcol_make(0, 1))],
            25: [lambda: v1_chunk(3)],
            26: [lambda: v1_chunk(4)],
            27: [lambda: v1_chunk(5)],
            28: [lambda: v1_chunk(6)],
            29: [lambda: v1_chunk(7)],
        }
        # U(0) pair j at unit 30+j; pden(0) pair j at unit 33+j (the et
        # ring frees an et only after its pden read, so pden must trail U
        # closely for the 36-deep ET ring); normalize(0) at 49/50; U/pden(1)
        # as soon as its et pair + the U banks are available.
        for j in range(16):
            fillers.setdefault(30 + j, []).append(
                lambda j=j: upden0_U_step(j))
        for j in range(16):
            fillers.setdefault(33 + j, []).append(
                lambda j=j: upden0_P_step(j))
        fillers.setdefault(49, []).append(lambda: normalize_A(0))
        fillers.setdefault(50, []).append(lambda: normalize_B(0))
        u1_sched = {51: [0, 1], 52: [2, 3], 53: [4, 5], 54: [6, 7],
                    55: [8, 9], 56: [10, 11], 57: [12], 60: [13], 62: [14]}
        for uu, js in u1_sched.items():
            for j in js:
                fillers.setdefault(uu, []).append(
                    lambda j=j: upden1_pair(j))

        for u in range(64):
            nh2 = u // 32
            r = u % 32
            ms, grp, pr = r // 4, (r // 2) % 2, r % 2
            et = scores_tile(nh2, ms, grp, pr)
            ets.setdefault((nh2, ms, grp), []).append(et)
            for f in fillers.get(u, ()):
                f()
        upden1_pair(15)
        normalize_proj(1)

        XG.release()
        ET.release()
    nc.finalize()
    return nc


def _get_program():
    if "nc" not in _prog_cache:
        _prog_cache["nc"] = _build_program()
    return _prog_cache["nc"]


def kernel(x1, x2, q_w, kv_w, sr_w, sr_b, ln_g, ln_b, proj_w, proj_b,
           H1=64, W1=64, H2=64, W2=64, **_):
    from concourse.bass_utils import run_bass_kernel_spmd

    f = np.float32
    x1 = np.asarray(x1, f)
    x2 = np.asarray(x2, f)
    q_w = np.asarray(q_w, f)
    kv_w = np.asarray(kv_w, f)
    sr_w = np.asarray(sr_w, f)
    sr_b = np.asarray(sr_b, f)
    ln_g = np.asarray(ln_g, f)
    ln_b = np.asarray(ln_b, f)
    proj_w = np.asarray(proj_w, f)
    proj_b = np.asarray(proj_b, f)

    import ml_dtypes
    bf = ml_dtypes.bfloat16

    qwT = np.ascontiguousarray(q_w.T * SCALE)
    kwTf = np.ascontiguousarray(ln_g[:, None] * kv_w[:C].T)   # [cin, out]
    vwTf = np.ascontiguousarray(ln_g[:, None] * kv_w[C:].T)
    kfcol_neg = -kwTf.sum(axis=0)    # [C]
    vfcol_neg = -vwTf.sum(axis=0)
    bvec_v = kv_w[C:] @ ln_b
    pbias = proj_b + proj_w @ bvec_v
    pwT = np.ascontiguousarray(proj_w.T)
    # [2, 128, kh, kw, C_out]: stationary[c, kh, kw, o] = sr_w[o, c, kh, kw]
    w2h = sr_w.transpose(1, 2, 3, 0).astype(bf)
    w2c = np.ascontiguousarray(
        np.stack([w2h[:128], w2h[128:]], axis=0))
    rowd = np.stack([kfcol_neg, vfcol_neg, pbias], axis=0)    # [3, C]
    cold = np.stack([sr_b[:128], sr_b[128:],
                     pbias[:128], pbias[128:]], axis=1)       # [128, 4]
    blkd = np.zeros((2, 128, 128), bf)
    for grp in range(2):
        for i in range(128):
            h = grp * 4 + i // 32
            src_row = 32 * (h % 4) + h // 4
            blkd[grp, src_row, i] = 1.0

    x1T = [np.ascontiguousarray(x1[b].T).astype(bf) for b in range(B)]
    x2T = [np.ascontiguousarray(x2[b].T).astype(bf) for b in range(B)]

    # Column-rotate x per core so its q rows are always x1 cols [0:1024).
    # A rotation by chk*1024 pixels = 16*chk image rows (even), which maps
    # the stride-2 conv's 2x2 blocks onto blocks, so conv/LN/k2/v1 see a
    # consistent permutation of m for both inputs; softmax-attention is
    # invariant to a shared permutation of the key/value axis.
    wallh = np.ascontiguousarray(
        np.concatenate([qwT, kwTf, vwTf, pwT], axis=1)).astype(bf)
    in_maps = []
    for core in range(8):
        b, chk = divmod(core, 4)
        s = chk * NCH
        in_maps.append({
            "x1t": np.ascontiguousarray(np.roll(x1T[b], -s, axis=1)),
            "x2t": np.ascontiguousarray(np.roll(x2T[b], -s, axis=1)),
            "w2": w2c,
            "wall": wallh,
            "rowd": rowd.astype(bf), "cold": cold.astype(np.float32),
            "blkd": blkd,
        })

    nc = _get_program()
    res = run_bass_kernel_spmd(nc, in_maps, core_ids=list(range(8)))
    _prog_cache["last_result"] = res
    out = np.empty((B, N, C), f)
    for core in range(8):
        b, chk = divmod(core, 4)
        out[b, chk * NCH:(chk + 1) * NCH, :] = res.results[core]["outt"].T
    return out
